# revision 1
# baseline (speedup 1.0000x reference)
"""Bass/Trainium2 kernel for nn_AttentionPooling2 (segment_reduce).

Math (per batch b):
    scores = gelu(LN(doc_state @ W1 + b1) * gamma + beta) @ W2 + b2      # (S,)
    logits = M * scores + (1-M) * (-1e4);  attn = softmax_S(logits)
    pooled = einsum('ns,ns,sd->nd', M, attn, doc_state)

Because M is binary and exp(-1e4 - max) underflows to exactly 0 in fp32,
the reference result collapses to
    pooled[n] = (M[n] * e) @ X / (M[n] @ e),   e = exp(scores)
(the softmax max-subtraction and b2 cancel in the ratio).  So per core we:
  1. h = X @ W1 on PE (lhsT = X^T built with PE is_transpose matmuls)
  2. LayerNorm stats via bn_stats; apply LN fused into the GELU
     activation (per-partition scale=rstd, bias=-mean*rstd)
  3. scores via DVE scalar_tensor_tensor + accum against broadcast W2
  4. e = exp(s) = (1+tanh(s/2))/(1-tanh(s/2)) -- tanh is in the gelu ACT
     table set, so the only mid-kernel table switch is the gelu load
  5. scale M^T by e (per-partition; half as wide as scaling X), then
     pooled num/den via accumulated PE matmuls against X and a ones col
  6. out = num * reciprocal(den + 1e-30)

All matmul operands are float32r (4x PE throughput at free dims >= 256,
~1e-4 relative rounding; every producer feeding a matmul emits f32r).

Sharding: pure data-parallel, batch b -> core b (B == 8 == n_cores).
M^T is pre-transposed on the host (numpy) so it needs no device transposes.
Built with Bacc (not raw Bass): its generate_event_semaphores pass splits
multi-waits to satisfy TRN2's one-sync-wait-per-instruction constraint.
"""

import os

import numpy as np

B, S, N, D = 8, 1024, 128, 256
P = 128          # partitions
ST = S // P      # 8 token tiles
DC = D // P      # 2 contraction chunks
LN_EPS = 1e-5

_CACHE = {}

USE_R32 = True    # float32r matmuls: 4x PE throughput at free-dim >= 256
MT_U8 = True      # ship the binary mask as uint8, cast during SWDGE DMA


def _build(fast_ln: bool):
    from contextlib import ExitStack

    import concourse.bass as bass
    import concourse.tile as tile
    from concourse import bacc, mybir
    from concourse.masks import make_identity

    f32 = mybir.dt.float32
    u8 = mybir.dt.uint8
    u32 = mybir.dt.uint32
    AF = mybir.ActivationFunctionType
    OP = mybir.AluOpType

    f32r = mybir.dt.float32r if USE_R32 else f32

    nc = bacc.Bacc("TRN2")
    x = nc.dram_tensor("x", [S, D], f32r, kind="ExternalInput")
    mt = nc.dram_tensor("mt", [S, N], u8 if MT_U8 else f32r,
                        kind="ExternalInput")
    w1 = nc.dram_tensor("w1", [P, 3, D], f32r, kind="ExternalInput")
    if not fast_ln:
        b1d = nc.dram_tensor("b1", [1, D], f32, kind="ExternalInput")
        gmd = nc.dram_tensor("gamma", [1, D], f32, kind="ExternalInput")
        btd = nc.dram_tensor("beta", [1, D], f32, kind="ExternalInput")
    out = nc.dram_tensor("out", [N, D], f32, kind="ExternalOutput")

    x_re = x.rearrange("(t p) d -> p t d", p=P)       # [128, 8, 256]
    mt_re = mt.rearrange("(t p) n -> p t n", p=P)     # [128, 8, 128]

    def bcast(handle):  # [1, D] dram -> [[0,P],[1,D]] broadcast AP
        return bass.AP(handle, 0, [[0, P], [1, D]])

    with tile.TileContext(nc) as tc, ExitStack() as ctx:
        consts = ctx.enter_context(tc.tile_pool(name="consts", bufs=1))
        big = ctx.enter_context(tc.tile_pool(name="big", bufs=1))
        xtp = ctx.enter_context(tc.tile_pool(name="xtp", bufs=3))
        gelu_p = ctx.enter_context(tc.tile_pool(name="gelu", bufs=3))
        scr_p = ctx.enter_context(tc.tile_pool(name="scr", bufs=2))
        stat_p = ctx.enter_context(tc.tile_pool(name="stat", bufs=2))
        ps_t = ctx.enter_context(tc.tile_pool(name="ps_t", bufs=1, space="PSUM"))
        ps_h = ctx.enter_context(tc.tile_pool(name="ps_h", bufs=2, space="PSUM"))
        ps_o = ctx.enter_context(tc.tile_pool(name="ps_o", bufs=1, space="PSUM"))

        ident_f = consts.tile([P, P], f32)
        make_identity(nc, ident_f)
        ident = ident_f
        if USE_R32:
            # memset can't write f32r; build in f32 then cast-copy once
            ident = consts.tile([P, P], f32r, tag="ident_r")
            nc.vector.tensor_copy(out=ident, in_=ident_f)
        eps_sb = consts.tile([P, 1], f32)
        nc.vector.memset(eps_sb, LN_EPS)
        # dummy sqrt so walrus preloads the sqrt table set at t=0 (overlaps
        # the input DMA); the xt copies run from it (copy is in every set),
        # the mid-kernel rstd sqrt then needs NO load, and the only paid
        # table switch left is the gelu set
        g_warm = consts.tile([1, 1], f32)
        nc.scalar.activation(out=g_warm, in_=eps_sb[0:1, :], func=AF.Sqrt)
        ones_f = consts.tile([P, 2], f32)
        nc.vector.memset(ones_f, 1.0)
        ones_r = consts.tile([P, 2], f32r)
        nc.vector.tensor_copy(out=ones_r, in_=ones_f)

        x_sb = big.tile([P, ST, D], f32r)
        mt_sb = big.tile([P, ST, N], f32r)
        # [c0|c1] = W1 contraction chunks, [2] = host-broadcast W2 row
        # (f32r is a bit-preserving view for non-PE consumers)
        w12_sb = big.tile([P, 3, D], f32r)
        w1_sb = w12_sb[:, 0:2, :]
        w2_sb = w12_sb[:, 2, :]
        # split the 1MB x load so compute can start on the first half early;
        # balance bytes across the SP HWDGE ring and the SWDGE path (the
        # ACT ring is kept free for compute).  The mask ships as uint8 and
        # is cast on GPSIMD (cast-DMA is slow), W2 arrives host-replicated.
        mt_u8sb = None
        if MT_U8:
            mt_u8sb = big.tile([P, ST, N], u8, tag="mt_u8sb")
        nc.sync.dma_start(out=x_sb[:, 0:1, :], in_=x_re[:, 0:1, :])
        nc.sync.dma_start(out=w12_sb[:, 0:1, :], in_=w1[:, 0:1, :])
        nc.sync.dma_start(out=x_sb[:, 1:4, :], in_=x_re[:, 1:4, :])
        nc.gpsimd.dma_start(out=x_sb[:, 4:5, :], in_=x_re[:, 4:5, :])
        nc.gpsimd.dma_start(out=x_sb[:, 5:8, :], in_=x_re[:, 5:8, :])
        nc.gpsimd.dma_start(out=w12_sb[:, 1:3, :], in_=w1[:, 1:3, :])
        if MT_U8:
            nc.sync.dma_start(out=mt_u8sb, in_=mt_re)
            nc.gpsimd.tensor_copy(out=mt_sb, in_=mt_u8sb)
        else:
            nc.gpsimd.dma_start(out=mt_sb, in_=mt_re)
        if not fast_ln:
            b1_sb = consts.tile([P, D], f32)
            gm_sb = consts.tile([P, D], f32)
            bt_sb = consts.tile([P, D], f32)
            nc.gpsimd.dma_start(out=b1_sb, in_=bcast(b1d))
            nc.gpsimd.dma_start(out=gm_sb, in_=bcast(gmd))
            nc.gpsimd.dma_start(out=bt_sb, in_=bcast(btd))

        s_col = consts.tile([P, ST], f32)   # scores, tile t in column t
        e_col = consts.tile([P, ST], f32)   # exp(scores)
        mv = consts.tile([P, ST, 2], f32)   # per-tile mean/var
        rstd = consts.tile([P, ST], f32)
        nmr = consts.tile([P, ST], f32)     # -mean * rstd

        phs = []
        for half in range(2):
            ts0 = 4 * half
            # X^T staging PSUM (2 banks): regions 2*tt+c written exactly once;
            # regions 0-3 = bank A (token tiles ts0, ts0+1), 4-7 = bank B
            pt = ps_t.tile([P, 8, P], f32r, tag="pt")
            ph = ps_h.tile([P, 4, D], f32, tag="ps_h")   # 2 PSUM banks
            phs.append(ph)
            for tt in range(4):
                t = ts0 + tt
                for c in range(DC):
                    nc.tensor.transpose(pt[:, 2 * tt + c, :],
                                        x_sb[:, t, c * P:(c + 1) * P],
                                        ident)
            for pair in range(2):
                # copy one full PSUM bank (2 token tiles) per op, alternating
                # between the ACT and DVE engines
                xt = xtp.tile([P, 4, P], f32r, tag="xt")
                nc.scalar.copy(out=xt, in_=pt[:, 4 * pair:4 * pair + 4, :])
                for i in range(2):
                    tt = 2 * pair + i
                    for c in range(DC):
                        nc.tensor.matmul(ph[:, tt, :],
                                         lhsT=xt[:, 2 * i + c, :],
                                         rhs=w1_sb[:, c, :],
                                         start=(c == 0), stop=(c == DC - 1))
            if not fast_ln:
                # h += b1 (general path only; b1 is zeros in this problem)
                for tt in range(4):
                    nc.vector.tensor_tensor(out=ph[:, tt, :], in0=ph[:, tt, :],
                                            in1=b1_sb, op=OP.add)
            # LayerNorm stats (bn_stats must be 2D: the AP optimizer collapses
            # contiguous group dims, which breaks grouped stats)
            stats = stat_p.tile([P, 4, 6], f32, tag="stats")
            for tt in range(4):
                nc.vector.bn_stats(out=stats[:, tt, :], in_=ph[:, tt, :])
                nc.vector.bn_aggr(out=mv[:, ts0 + tt, :], in_=stats[:, tt, :])

        # rstd = 1/sqrt(var+eps), both halves in ONE batch.  This rides the
        # half-B dependency chain (the critical path) so it costs nothing
        # extra, and avoids loading the sqrt table set twice.
        nc.scalar.activation(out=rstd, in_=mv[:, :, 1], func=AF.Sqrt,
                             bias=eps_sb, scale=1.0)
        nc.vector.reciprocal(out=rstd, in_=rstd)
        nc.vector.scalar_tensor_tensor(out=nmr, in0=mv[:, :, 0], scalar=-1.0,
                                       in1=rstd, op0=OP.mult, op1=OP.mult)
        for t in range(ST):
            ph = phs[t // 4]
            tt = t % 4
            g_t = gelu_p.tile([P, D], f32, tag="gelu")
            if fast_ln:
                # gelu(h*rstd - mean*rstd) straight out of PSUM
                nc.scalar.activation(out=g_t, in_=ph[:, tt, :], func=AF.Gelu,
                                     scale=rstd[:, t:t + 1],
                                     bias=nmr[:, t:t + 1])
            else:
                xh = gelu_p.tile([P, D], f32, tag="xh")
                nc.vector.tensor_scalar(out=xh, in0=ph[:, tt, :],
                                        scalar1=mv[:, t, 0:1],
                                        scalar2=rstd[:, t:t + 1],
                                        op0=OP.subtract, op1=OP.mult)
                nc.vector.scalar_tensor_tensor(out=xh, in0=xh, scalar=1.0,
                                               in1=gm_sb, op0=OP.mult,
                                               op1=OP.mult)
                nc.vector.tensor_tensor(out=xh, in0=xh, in1=bt_sb, op=OP.add)
                nc.scalar.activation(out=g_t, in_=xh, func=AF.Gelu)
            # score_t = sum_d g_t * W2 (b2 cancels in the ratio); alternate
            # DVE / GPSIMD.  (tensor_tensor_reduce is a custom ANT DVE op
            # that faults on this runtime path; scalar_tensor_tensor works.)
            sc = scr_p.tile([P, D], f32, tag="scr")
            nc.vector.scalar_tensor_tensor(out=sc, in0=g_t, scalar=1.0,
                                           in1=w2_sb, op0=OP.bypass,
                                           op1=OP.mult,
                                           accum_out=s_col[:, t:t + 1])


        # e^s = (1+tanh(s/2)) / (1-tanh(s/2)): tanh is in the gelu table
        # set (no exp-set load), and each half is converted as soon as its
        # scores exist so the pooled chain starts early.
        # Keep the tensor engine continuously busy from rstd-time until the
        # pooled chain starts: back-to-back dummy matmuls (complete groups
        # into po[0:8,:], fully overwritten by the real start=True chains)
        # hold the PE ramp/HAM at full clock so the pooled matmuls run ~2x
        # faster.  po is read at the end, so Bacc DCE keeps them.
        xf = x_sb.bitcast(f32)
        po = ps_o.tile([P, D + 2], f32)
        for _ in range(11):
            nc.tensor.matmul(po[0:8, 0:D], lhsT=rstd[:, 0:8],
                             rhs=xf[:, 0, 0:D],
                             start=True, stop=True, skip_group_check=True)

        th = consts.tile([P, ST], f32)
        e_den = consts.tile([P, ST], f32)
        mts = big.tile([P, ST, N], f32r)
        for half in range(2):
            hs = bass.ds(4 * half, 4)
            nc.scalar.activation(out=th[:, hs], in_=s_col[:, hs],
                                 func=AF.Tanh, scale=0.5)
            nc.vector.tensor_scalar(out=e_den[:, hs], in0=th[:, hs],
                                    scalar1=-1.0, scalar2=1.0,
                                    op0=OP.mult, op1=OP.add)
            nc.vector.reciprocal(out=e_den[:, hs], in_=e_den[:, hs])
            nc.vector.scalar_tensor_tensor(out=e_col[:, hs], in0=th[:, hs],
                                           scalar=1.0, in1=e_den[:, hs],
                                           op0=OP.add, op1=OP.mult)
            for tt in range(4):
                t = 4 * half + tt
                eng = nc.vector if t % 2 == 0 else nc.gpsimd
                eng.tensor_scalar_mul(out=mts[:, t, :], in0=mt_sb[:, t, :],
                                      scalar1=e_col[:, t:t + 1])

        for t in range(ST):
            nc.tensor.matmul(po[:, 0:D], lhsT=mts[:, t, :], rhs=x_sb[:, t, :],
                             start=(t == 0), stop=(t == ST - 1))
        for t in range(ST):
            nc.tensor.matmul(po[:, D:D + 2], lhsT=mts[:, t, :], rhs=ones_r,
                             start=(t == 0), stop=(t == ST - 1))

        dinv = consts.tile([P, 1], f32)
        nc.vector.tensor_scalar_add(out=dinv, in0=po[:, D:D + 1], scalar1=1e-30)
        nc.vector.reciprocal(out=dinv, in_=dinv)
        out_sb = big.tile([P, D], f32)
        nc.vector.tensor_scalar_mul(out=out_sb, in0=po[:, 0:D], scalar1=dinv)
        nc.sync.dma_start(out=out[:, :], in_=out_sb)

    nc.compile()
    _check_wait_counts(nc)
    return nc


def _check_wait_counts(nc):
    """TRN2 allows one sync wait per instruction (two on InstEventSemaphore);
    Bacc's generate_event_semaphores should guarantee this — verify."""
    import json

    m = json.loads(nc.to_json_bytes())
    bad = []
    for f in m["functions"]:
        for blk in f["blocks"]:
            for ins in blk["instructions"]:
                op = str(ins.get("opcode", ""))
                waits = (ins.get("sync_info") or {}).get("on_wait") or []
                limit = 2 if ("EventSemaphore" in op or "Drain" in op) else 1
                if len(waits) > limit:
                    bad.append((ins.get("name"), op,
                                [(w.get("ant_name"), w.get("wait_value"))
                                 for w in waits]))
    if bad:
        raise AssertionError(f"instructions over the wait limit: {bad}")


def kernel(doc_state, nodes_mapping, nodes_len, W1, b1, gamma, beta, W2, b2,
           _trace=False):
    from concourse.bass_utils import run_bass_kernel_spmd

    doc_state = np.ascontiguousarray(doc_state, dtype=np.float32)
    nodes_mapping = np.asarray(nodes_mapping, dtype=np.float32)
    W1 = np.asarray(W1, dtype=np.float32)
    # pack [W1 chunk0 | W1 chunk1 | broadcast W2 row] as one [P, 3, D] DMA
    w12 = np.stack([W1[0:P], W1[P:2 * P],
                    np.broadcast_to(np.asarray(W2, np.float32).reshape(1, D),
                                    (P, D))], axis=1)
    w12 = np.ascontiguousarray(w12)
    b1 = np.asarray(b1, dtype=np.float32).reshape(-1)
    gamma = np.asarray(gamma, dtype=np.float32).reshape(-1)
    beta = np.asarray(beta, dtype=np.float32).reshape(-1)

    fast_ln = (not b1.any()) and bool(np.all(gamma == 1.0)) and (not beta.any())
    key = ("nc", fast_ln)
    if key not in _CACHE:
        _CACHE[key] = _build(fast_ln)
    nc = _CACHE[key]

    # host-side prep: transpose the binary mask so the device needs no
    # M transposes (M only ever enters matmuls contracted over S); ship it
    # as uint8 (4x less DMA) and let SWDGE cast to f32 on the way in
    mt_all = np.ascontiguousarray(nodes_mapping.transpose(0, 2, 1))
    if MT_U8:
        mt_all = mt_all.astype(np.uint8)

    in_maps = []
    for b in range(B):
        m = {"x": doc_state[b], "mt": mt_all[b], "w1": w12}
        if not fast_ln:
            m["b1"] = b1.reshape(1, D)
            m["gamma"] = gamma.reshape(1, D)
            m["beta"] = beta.reshape(1, D)
        in_maps.append(m)

    res = run_bass_kernel_spmd(nc, in_maps, core_ids=list(range(B)),
                               trace=_trace)
    out = np.stack([res.results[b]["out"] for b in range(B)], axis=0)
    if _trace:
        kernel.last_exec_time_ns = res.exec_time_ns
        kernel.last_trace = res.instructions_and_trace
    return out

